# revision 65
# baseline (speedup 1.0000x reference)
"""Multi-head causal attention (B=2, T=2048, C=1024, H=16, D=64) on 8 TRN2 cores.

Sharding: 2 heads per core (tensor-parallel over H). x is replicated (passed
pre-transposed as x^T so the contraction dim lands on SBUF partitions). Each
core computes y[:, :, 2c*64:(2c+2)*64]; host concatenates along channels.

Per-core dataflow (bf16 matmuls at full PE rate; Q/K projections in fp8
DoubleRow -- 2 fp8 weights per PE cell halves their column-write cycles;
PSUM accumulation stays fp32):
  1. Projections, W stationary -> Q^T/K^T/V^T in [dd=2*64, t] layout (both
     heads stacked on partitions). Q/K use fp8 e4m3 x^T and W repacked on
     host into the DoubleRow [K,2,free] pair layout, W pre-scaled by 64 so
     fp8 has mantissa headroom (raw W ~ N(0, 0.02) underflows e4m3); the
     1/sqrt(C)/64^2 compensation rides the exp() activation's scale arg.
     V uses bf16 (fp8 V error is exposed raw by early rows' tiny softmax).
  2. V^T PE-transposed (bf16) to V[s, d] per head, stored as V_aug[s, 80] =
     [ones | V | zeros] so the AV matmul emits softmax sums in PSUM row 0.
  3. Scores S^T[s, t] = K^T(stationary) x Q^T(moving) per s-block, both
     heads' matmuls in one 2-bank PSUM tile on DISJOINT 64-partition
     contraction rows -> the PE runs them concurrently as 64x128 row
     tiles; columns below the causal diagonal are never computed.
  4. ONE exp call per s-block covers both heads PSUM->SBUF (bf16 out, no
     max-subtraction needed: |scores| <= ~1; trimmed ranges use a single
     strided 3D-AP call). The diagonal 128x128 gets a multiplicative 0/1
     triangle on DVE, off the ScalarE critical path.
  5. AV: V_aug stationary, E^T moving (N trimmed), accumulated over
     s-blocks in PSUM -> out^T[80, t] (row 0 = softmax sums).
  6. Drain to bf16, PE-transpose each 128-col block to [t, 80], DVE
     reciprocal of the sums column + per-partition scalar multiply in
     f32, DMA out. This epilogue is DEFERRED as fine-grained closures
     drained during the NEXT tile's periods -- emitted inline it parks
     the next tile's score matmuls behind the ot-transpose waits in the
     Tensor FIFO at every tile boundary (~2.5us total). The final tile
     pipelines its epilogue by column-QUARTER inside the period loop:
     each av_ps quarter is drained/transposed/stored as soon as its last
     contributing s-block lands (causal trim), split across ScalarE and
     DVE with alternating y-DMA queues -- the tail keeps only the last
     quarter's chain.

Schedule: one fused streaming pipeline per (b, t-tile); the NEXT tile's
projection work is emitted in closures interleaved between the current
tile's attention s-block periods, and AV lags scores by 2 s-blocks, so the
PE never idles (keeps the HAM clock gate at K=8/8) and ScalarE never
starves at tile boundaries. W loads are the first dispatches on the GpSimd
DMA queue (earliest preamble) so the first projection matmul is gated only
by wq + x^T chunk 0.

Rejected experiments (measured slower or failed the 2e-2 absmax gate):
XBAR DMA transposes for the epilogue (1.2us/dispatch chokes the Sync
queue); fp8 DoubleRow AV over s-block pairs, both barrier-scheduled and
de-lumped one-head-per-period (PE busy stayed ~113us -- the PE is issue-
rate-limited at ~80ns/matmul, so cutting streaming cycles without cutting
instruction count gains nothing, while the pair coupling adds stalls);
fp8 V or fp8 E globally (early rows expose raw quantization error,
absmax 3e-2); GpSimd affine_select causal mask (slower than the DVE
multiply); LAG=3; batch-interleaved tile order; per-chunk W loads
(consistently +4us: extra gpsimd dispatches delay the startup chain);
splitting every tile's epilogue onto ScalarE (delays mid-run exp);
fp16 E instead of bf16 (time-neutral, slightly worse error);
epilogue closures ordered before the next x-tile load (+1us);
per-period AV emitted before the pending-closure drain (neutral).
"""

import numpy as np

import concourse.mybir as mybir
import concourse.tile as tile
from concourse import bacc
from concourse.masks import make_identity

B, T, C, H, D = 2, 2048, 1024, 16, 64
HPC = 2          # heads per core
NCORES = 8
TT = 512         # t-tile (moving free dim)
SB = 128         # s-block (scores stationary free dim)
NCH = C // 128   # contraction chunks for projections
NCH2 = C // 256  # DoubleRow chunks (256 contraction rows each)
VW = 80          # V_aug width: [ones | V (64) | zeros (15)]
F32 = mybir.dt.float32
BF16 = mybir.dt.bfloat16
FP8 = mybir.dt.float8e4
FP16 = mybir.dt.float16
# Wq/Wk are pre-scaled by WPRE on host so their fp8 encoding has headroom
# (raw W ~ N(0, 0.02) sits at the bottom of e4m3's range); the exp()
# activation's scale argument compensates with C**-0.5 / WPRE**2.
WPRE = 64.0
EXP_SCALE = float(C) ** -0.5 / (WPRE * WPRE)


def build_nc(t_len=T, batches=B):
    nj = t_len // TT
    nc = bacc.Bacc("TRN2", target_bir_lowering=False, debug=False)
    xt = nc.dram_tensor("xt", [batches, C, t_len], BF16, kind="ExternalInput")
    # x^T repacked for DoubleRow: (p, kk, r, t) = x[b, t, kk*256 + r*128 + p]
    xt8 = nc.dram_tensor("xt8", [batches, 128, NCH2, 2, t_len], FP8,
                         kind="ExternalInput")
    # W repacked for DoubleRow: (p, kk, r, m) = W[kk*256 + r*128 + p, m]
    wq = nc.dram_tensor("wq", [128, NCH2, 2, 2 * D], FP8, kind="ExternalInput")
    wk = nc.dram_tensor("wk", [128, NCH2, 2, 2 * D], FP8, kind="ExternalInput")
    wv = nc.dram_tensor("wv", [C, 2 * D], BF16, kind="ExternalInput")
    y = nc.dram_tensor("y", [batches, t_len, 2 * D], F32, kind="ExternalOutput")

    with tile.TileContext(nc) as tc:
        with (
            tc.tile_pool(name="consts", bufs=1) as consts,
            tc.tile_pool(name="wpool", bufs=1) as wpool,
            tc.tile_pool(name="qkv", bufs=batches) as qkv,
            tc.tile_pool(name="epool", bufs=4) as epool,
            tc.tile_pool(name="avs", bufs=4) as avs,
            tc.tile_pool(name="outp", bufs=8) as outp,
            tc.tile_pool(name="small", bufs=16) as small,
        ):
            w_sb, w_src = {}, {}
            for name, w in (("q", wq), ("k", wk)):
                wt = wpool.tile([128, NCH2, 2, 2 * D], FP8, tag=f"w{name}",
                                name=f"w{name}_sb")
                w_sb[name] = wt
                w_src[name] = w
            wv_sb = wpool.tile([128, NCH, 2 * D], BF16, tag="wv", name="wv_sb")
            w_sb["v"] = wv_sb
            w_src["v"] = wv

            # W loads go FIRST on the GpSimd DMA queue -- its framework
            # preamble finishes earliest, so wq is in flight ~1us before the
            # Sync queue can even dispatch. (The consts' memsets below would
            # otherwise sit ahead of these dispatches in the gpsimd FIFO.)
            # (Moving wq to the Scalar queue, whose first DMA dispatch lands
            # ~0.6us earlier per the trace, measured no difference.)
            for name in ("q", "k"):
                nc.gpsimd.dma_start(out=w_sb[name], in_=w_src[name][:])
            nc.scalar.dma_start(
                out=w_sb["v"],
                in_=w_src["v"].rearrange("(k p) d -> p k d", p=128))

            identity = consts.tile([128, 128], BF16)
            make_identity(nc, identity)
            # tri01[s, t_local] = 1 where t_local >= s else 0; multiplied
            # into the diagonal 128x128 sub-block of E after exp.
            tri01 = consts.tile([128, SB], BF16)
            nc.gpsimd.memset(tri01, 1.0)
            nc.gpsimd.affine_select(
                out=tri01, in_=tri01,
                compare_op=mybir.AluOpType.is_ge,
                fill=0.0, base=0,
                pattern=[[1, SB]], channel_multiplier=-1,
            )

            # Persistent per-batch tensors
            QT, KT, VHB = {}, {}, {}
            for b in range(batches):
                QT[b] = qkv.tile([128, t_len], BF16, tag="qt", name=f"qt{b}")
                KT[b] = qkv.tile([128, t_len], BF16, tag="kt", name=f"kt{b}")
                for h in range(HPC):
                    vhb = qkv.tile([128, (t_len // SB) * VW], BF16,
                                   tag=f"vhb{h}", name=f"vhb{b}_{h}")
                    vbv = vhb.rearrange("p (i c) -> p i c", c=VW)
                    nc.gpsimd.memset(vbv[:, :, 0:1], 1.0)
                    nc.gpsimd.memset(vbv[:, :, D + 1:VW], 0.0)
                    VHB[(b, h)] = vhb

            # ---------------- fused streaming pipeline ----------------
            # Per (b, j): projections for t-tile j, then causal attention for
            # t-tile j (which only needs K/V up to tile j). One PSUM budget,
            # no phase boundary, so the PE stays continuously busy and the
            # HAM clock-gate stays warm. The attention inner loop software-
            # pipelines two head-streams with lag-1 AV so the PE never
            # stalls on exp.
            with (
                tc.tile_pool(name="xtp", bufs=3) as xtp,
                tc.tile_pool(name="vts", bufs=2) as vts,
                tc.tile_pool(name="mixps", bufs=2, space="PSUM") as mixps,
                tc.tile_pool(name="spsum", bufs=2, space="PSUM") as spsum,
                tc.tile_pool(name="avpsum", bufs=2, space="PSUM") as avpsum,
            ):
                def proj_closures(b, j, chunked=False):
                    """Projection work for (b, j) as a list of closures, to
                    be interleaved into the previous tile's attention
                    periods so neither PE nor ScalarE ever starves."""
                    state = {}

                    def do_load():
                        x8r = xt8[b]
                        xt8_sb = xtp.tile([128, NCH2, 2, TT], FP8, tag="xt8",
                                          name=f"xt8_{b}_{j}")
                        xr = xt[b].rearrange("(k p) t -> p k t", p=128)
                        xt_sb = xtp.tile([128, NCH, TT], BF16, tag="xts",
                                         name=f"xts{b}_{j}")
                        if chunked:
                            # per-chunk DMAs so the first proj matmul only
                            # waits for chunk 0 (cuts pipeline-fill)
                            for kk in range(NCH2):
                                nc.sync.dma_start(
                                    out=xt8_sb[:, kk],
                                    in_=x8r[:, kk, :, j * TT:(j + 1) * TT])
                            for kk in range(NCH):
                                nc.sync.dma_start(
                                    out=xt_sb[:, kk, :],
                                    in_=xr[:, kk, j * TT:(j + 1) * TT])
                        else:
                            nc.sync.dma_start(
                                out=xt8_sb,
                                in_=x8r[:, :, :, j * TT:(j + 1) * TT])
                            half = NCH // 2
                            nc.sync.dma_start(
                                out=xt_sb[:, 0:half, :],
                                in_=xr[:, 0:half, j * TT:(j + 1) * TT])
                            nc.sync.dma_start(
                                out=xt_sb[:, half:, :],
                                in_=xr[:, half:, j * TT:(j + 1) * TT])
                        state["xt8"] = xt8_sb
                        state["xt"] = xt_sb

                    def do_proj(name):
                        pp = mixps.tile([128, TT], F32, tag="mix",
                                        name=f"pp_{name}")
                        if name in ("q", "k"):
                            # fp8 DoubleRow: 256 contraction rows per matmul
                            # (2 fp8 weights per PE cell), half the column-
                            # write cycles of the bf16 path.
                            for kk in range(NCH2):
                                nc.tensor.matmul(
                                    pp,
                                    lhsT=w_sb[name][:, kk],
                                    rhs=state["xt8"][:, kk],
                                    start=(kk == 0), stop=(kk == NCH2 - 1),
                                    perf_mode=mybir.MatmulPerfMode.DoubleRow,
                                    skip_group_check=True,
                                )
                        else:
                            for kk in range(NCH):
                                nc.tensor.matmul(
                                    pp,
                                    lhsT=w_sb[name][:, kk, :],
                                    rhs=state["xt"][:, kk, :],
                                    start=(kk == 0), stop=(kk == NCH - 1),
                                    skip_group_check=True,
                                )
                        if name == "q":
                            nc.vector.tensor_copy(
                                QT[b][:, j * TT:(j + 1) * TT], pp)
                        elif name == "k":
                            nc.vector.tensor_copy(
                                KT[b][:, j * TT:(j + 1) * TT], pp)
                        else:
                            vt_sb = vts.tile([128, TT], BF16, tag="vt",
                                             name=f"vt{b}_{j}")
                            nc.vector.tensor_copy(vt_sb, pp)
                            state["vt"] = vt_sb

                    def do_vtrans(q4):
                        vp = mixps.tile([128, 128], BF16, tag="mix",
                                        name=f"vp{q4}")
                        nc.tensor.transpose(
                            vp, state["vt"][:, q4 * 128:(q4 + 1) * 128],
                            identity)
                        sb = (j * TT) // SB + q4
                        for h in range(HPC):
                            nc.vector.tensor_copy(
                                VHB[(b, h)][:, sb * VW + 1:sb * VW + 1 + D],
                                vp[:, h * D:(h + 1) * D])

                    ops = [lambda: do_proj("q"),
                           lambda: do_proj("k"),
                           lambda: do_proj("v")]
                    ops += [lambda q4=q4: do_vtrans(q4)
                            for q4 in range(TT // 128)]
                    return do_load, ops

                def emit_attention(b, j, pending, last=False):
                    """Causal attention for t-tile j. Per s-block: both
                    heads' score MMs into one paired PSUM tile [h0 | h1]
                    (disjoint row groups -> concurrent), ONE exp call for
                    both heads, multiplicative tri-mask on E after exp (off
                    the ACT critical path), AV lagging 2 s-blocks. Closures
                    in `pending` (next tile's projections) are drained
                    evenly across the periods."""
                    out_tiles = [outp.tile([128, 2 * D], F32, tag="out",
                                           name=f"out{b}_{j}_{q}")
                                 for q in range(TT // 128)]
                    n_sb = (j + 1) * TT // SB
                    av_ps = {h: avpsum.tile([VW, TT], F32, tag="avps",
                                            name=f"avps{h}")
                             for h in range(HPC)}
                    eg = {}
                    LAG = 2

                    def emit_scores(sb):
                        # off: columns below the causal diagonal are never
                        # computed (scores, exp, AV all trimmed to t >= s).
                        off = max(0, (sb - 4 * j) * SB)
                        S = spsum.tile([128, HPC * TT], F32,
                                       tag="spsum", name=f"s{sb}")
                        for h in range(HPC):
                            hp = slice(h * D, (h + 1) * D)
                            nc.tensor.matmul(
                                S[:, h * TT + off:(h + 1) * TT],
                                lhsT=KT[b][hp, sb * SB:(sb + 1) * SB],
                                rhs=QT[b][hp, j * TT + off:(j + 1) * TT],
                                start=True, stop=True,
                            )
                        e = epool.tile([128, HPC * TT], BF16, tag="e",
                                       name=f"e{sb}")
                        if off == 0:
                            nc.scalar.activation(
                                out=e, in_=S, scale=EXP_SCALE,
                                func=mybir.ActivationFunctionType.Exp)
                        else:
                            # one strided call covers both heads' trimmed
                            # column ranges (saves the per-call ACT overhead)
                            ev = e.rearrange("p (h t) -> p h t", h=HPC)
                            Sv = S.rearrange("p (h t) -> p h t", h=HPC)
                            nc.scalar.activation(
                                out=ev[:, :, off:], in_=Sv[:, :, off:],
                                scale=EXP_SCALE,
                                func=mybir.ActivationFunctionType.Exp)
                        if sb >= 4 * j:  # diagonal triangle at cols [off, off+SB)
                            for h in range(HPC):
                                nc.vector.tensor_mul(
                                    e[:, h * TT + off:h * TT + off + SB],
                                    e[:, h * TT + off:h * TT + off + SB],
                                    tri01)
                        eg[sb] = (e, off)

                    def emit_av(sb):
                        e, off = eg.pop(sb)
                        for h in range(HPC):
                            nc.tensor.matmul(
                                av_ps[h][:, off:],
                                lhsT=VHB[(b, h)][:, sb * VW:(sb + 1) * VW],
                                rhs=e[:, h * TT + off:(h + 1) * TT],
                                start=(sb == 0), stop=(sb == n_sb - 1),
                                skip_group_check=True,
                            )

                    def epi_drain(h):
                        use_act = last and h == 1
                        av_sb = avs.tile([VW, TT], BF16, name=f"avsb{h}")
                        if use_act:
                            nc.scalar.copy(av_sb, av_ps[h])
                        else:
                            nc.vector.tensor_copy(av_sb, av_ps[h])
                        av_sbs[h] = av_sb

                    def epi_block(h, q4):
                        # PE transpose [80, 128] -> [128, 80] (bf16):
                        # col 0 = softmax sums, cols 1:65 = values.
                        use_act = last and h == 1
                        av_sb = av_sbs[h]
                        ot = mixps.tile([128, VW], BF16, tag="mix",
                                        name=f"ot{h}_{q4}")
                        nc.tensor.transpose(
                            ot, av_sb[:, q4 * 128:(q4 + 1) * 128],
                            identity[0:VW, 0:VW])
                        rec = small.tile([128, 1], F32)
                        nc.vector.reciprocal(rec, ot[:, 0:1])
                        if use_act:
                            nc.scalar.mul(
                                out_tiles[q4][:, h * D:(h + 1) * D],
                                ot[:, 1:1 + D], rec)
                        else:
                            nc.vector.tensor_scalar_mul(
                                out_tiles[q4][:, h * D:(h + 1) * D],
                                ot[:, 1:1 + D], rec)

                    def epi_store(q4):
                        t0 = j * TT + q4 * 128
                        eng = nc.scalar if (last and q4 % 2) else nc.sync
                        eng.dma_start(
                            out=y[b, t0:t0 + 128, :], in_=out_tiles[q4])


                    n_periods = n_sb + LAG
                    n_pend = len(pending)
                    popped = 0
                    av_sbs = {}
                    if last:
                        for h in range(HPC):
                            av_sbs[h] = avs.tile([VW, TT], BF16,
                                                 name=f"avsbL{h}")
                    for sb in range(n_periods):
                        if sb < n_sb:
                            emit_scores(sb)
                        want = (n_pend * (sb + 1)) // n_periods
                        while popped < want:
                            pending[popped]()
                            popped += 1
                        if sb >= LAG:
                            t_av = sb - LAG
                            emit_av(t_av)
                            if last and t_av >= n_sb - 4:
                                # Final tile: each av_ps column-quarter is
                                # complete as soon as its last contributing
                                # s-block lands (causal trim), so drain +
                                # transpose + normalize + store it NOW --
                                # the tail keeps only the last quarter's
                                # chain instead of the whole epilogue.
                                q = t_av - (n_sb - 4)
                                cs = slice(q * 128, (q + 1) * 128)
                                for h in range(HPC):
                                    if h == 1:
                                        nc.scalar.copy(av_sbs[h][:, cs],
                                                       av_ps[h][:, cs])
                                    else:
                                        nc.vector.tensor_copy(
                                            av_sbs[h][:, cs],
                                            av_ps[h][:, cs])
                                    epi_block(h, q)
                                epi_store(q)
                    assert popped == n_pend

                    # Output epilogue as CLOSURES: emitting these inline
                    # would park the next tile's score matmuls behind the
                    # ot-transpose dependency waits in the Tensor FIFO at
                    # every tile boundary. Instead they're returned and
                    # drained during the NEXT tile's periods (same trick as
                    # the projections). Only the final tile emits inline,
                    # with the chains split across DVE/ScalarE.

                    if last:
                        # Epilogue was already emitted in-loop, pipelined by
                        # column-quarter (see the period loop above).
                        return []
                    epi = []
                    for h in range(HPC):
                        epi.append(lambda h=h: epi_drain(h))
                        epi += [lambda h=h, q4=q4: epi_block(h, q4)
                                for q4 in range(TT // 128)]
                    epi += [lambda q4=q4: epi_store(q4)
                            for q4 in range(TT // 128)]
                    return epi

                seq = [(b, j) for b in range(batches) for j in range(nj)]
                ld0, ops0 = proj_closures(*seq[0], chunked=True)
                ld0()
                for op in ops0:
                    op()
                epi_prev = []
                for idx, (b, j) in enumerate(seq):
                    if idx + 1 < len(seq):
                        ldn, opsn = proj_closures(*seq[idx + 1])
                        nxt = [ldn] + epi_prev + opsn
                    else:
                        nxt = epi_prev
                    epi_prev = emit_attention(
                        b, j, nxt, last=(idx == len(seq) - 1))

    nc.compile()
    return nc


_CACHE = {}


def _get_runner():
    if "run" in _CACHE:
        return _CACHE["run"]

    import jax
    from jax.experimental.shard_map import shard_map
    from jax.sharding import Mesh, PartitionSpec
    from concourse import bass2jax
    from concourse.bass2jax import _bass_exec_p, install_neuronx_cc_hook

    nc = build_nc()
    install_neuronx_cc_hook()

    partition_name = (nc.partition_id_tensor.name
                      if nc.partition_id_tensor else None)
    in_names, out_names, out_avals, zero_outs = [], [], [], []
    for alloc in nc.m.functions[0].allocations:
        if not isinstance(alloc, mybir.MemoryLocationSet):
            continue
        name = alloc.memorylocations[0].name
        if alloc.kind == "ExternalInput":
            if name != partition_name:
                in_names.append(name)
        elif alloc.kind == "ExternalOutput":
            out_names.append(name)
            shape = tuple(alloc.tensor_shape)
            dtype = mybir.dt.np(alloc.dtype)
            out_avals.append(jax.core.ShapedArray(shape, dtype))
            zero_outs.append(np.zeros(shape, dtype))
    n_params = len(in_names)
    n_outs = len(out_avals)
    all_names = in_names + out_names
    if partition_name is not None:
        all_names = all_names + [partition_name]
    donate = tuple(range(n_params, n_params + n_outs))

    def _body(*args):
        operands = list(args)
        if partition_name is not None:
            operands.append(bass2jax.partition_id_tensor())
        outs = _bass_exec_p.bind(
            *operands,
            out_avals=tuple(out_avals),
            in_names=tuple(all_names),
            out_names=tuple(out_names),
            lowering_input_output_aliases=(),
            sim_require_finite=True,
            sim_require_nnan=True,
            nc=nc,
        )
        return tuple(outs)

    devices = jax.devices()[:NCORES]
    mesh = Mesh(np.asarray(devices), ("core",))
    in_specs = (PartitionSpec("core"),) * (n_params + n_outs)
    out_specs = (PartitionSpec("core"),) * n_outs
    sharded = jax.jit(
        shard_map(_body, mesh=mesh, in_specs=in_specs, out_specs=out_specs,
                  check_rep=False),
        donate_argnums=donate, keep_unused=True,
    )

    runner = {
        "nc": nc,
        "all_names": all_names,
        "sharded": sharded,
        "in_names": in_names,
        "out_names": out_names,
        "out_avals": out_avals,
        "zero_outs": zero_outs,
    }
    _CACHE["run"] = runner
    return runner


def _pack_w8(W2):
    """[C, 2D] f32 -> [128, NCH2, 2, 2D] fp8 DoubleRow layout, pre-scaled."""
    fp8 = mybir.dt.np(FP8)
    w = (W2 * WPRE).reshape(NCH2, 2, 128, 2 * D).transpose(2, 0, 1, 3)
    return np.ascontiguousarray(w).astype(fp8)


def _shard_inputs(x, Wq, Wk, Wv):
    """Per-core input dicts. Host-side layout prep only."""
    bf16 = mybir.dt.np(BF16)
    fp8 = mybir.dt.np(FP8)
    xt_f = np.transpose(x, (0, 2, 1))  # [B, C, T]
    xt = np.ascontiguousarray(xt_f).astype(bf16)
    # DoubleRow x^T: (b, p, kk, r, t) = x[b, t, kk*256 + r*128 + p]
    xt8 = np.ascontiguousarray(
        xt_f.reshape(B, NCH2, 2, 128, T).transpose(0, 3, 1, 2, 4)).astype(fp8)
    maps = []
    for c in range(NCORES):
        h0 = HPC * c
        wq2 = _pack_w8(np.concatenate([Wq[h0 + i] for i in range(HPC)], axis=1))
        wk2 = _pack_w8(np.concatenate([Wk[h0 + i] for i in range(HPC)], axis=1))
        wv2 = np.ascontiguousarray(
            np.concatenate([Wv[h0 + i] for i in range(HPC)], axis=1)).astype(bf16)
        maps.append({"xt": xt, "xt8": xt8, "wq": wq2, "wk": wk2, "wv": wv2})
    return maps


def run_sharded(in_maps):
    """Run the 8-core NEFF once; returns list of per-core output dicts."""
    r = _get_runner()
    concat_in = [
        np.concatenate([in_maps[c][name] for c in range(NCORES)], axis=0)
        for name in r["in_names"]
    ]
    concat_zeros = [
        np.zeros((NCORES * z.shape[0], *z.shape[1:]), z.dtype)
        for z in r["zero_outs"]
    ]
    out_arrs = r["sharded"](*concat_in, *concat_zeros)
    return [
        {
            name: np.asarray(out_arrs[i]).reshape(
                NCORES, *r["out_avals"][i].shape)[c]
            for i, name in enumerate(r["out_names"])
        }
        for c in range(NCORES)
    ]


def kernel(x, Wq, Wk, Wv):
    in_maps = _shard_inputs(
        np.asarray(x, dtype=np.float32), np.asarray(Wq, dtype=np.float32),
        np.asarray(Wk, dtype=np.float32), np.asarray(Wv, dtype=np.float32))
    results = run_sharded(in_maps)
    return np.concatenate([results[c]["y"] for c in range(NCORES)], axis=2)
